# revision 4
# baseline (speedup 1.0000x reference)
"""Trainium2 Bass kernel for nn_EuclideanLoss.

Math (matches the oracle):
    y_t  = transpose(y, (0, 2, 1))                 # [B, N, D]
    pd   = sqrt(sum((x - y_t)^2, axis=-1))         # [B, N]
    dist = mean(pd, axis=0); dist[1:3] *= 1.5
    loss = mean(dist)

Data-parallel over batch: each of the 8 NeuronCores takes 4 batches,
computes pd[b, n] on device (stored bf16); the tiny [B, N] result is
gathered to the host, which finishes mean/scale/mean in float64.

The problem is DMA-bound (16MB of input per core, ~42us at the per-core
HBM limit).  Trace-driven design (v11):

    n = nh*4096 + q*32 + c      q = partition, nh = n-half, c in [0,32)

DMA plan (all input DMAs issued before the prologue ops):
  y[b]: ONE flat DMA into [128, 4096], partition 2d+nh (16KB descriptors)
  x[b]: c-chunk DMAs matching the compute chunks:
        b0 [8,8,16]  - the first sub gates on only 8 transposes + 0.5MB
                       of x, starting DVE ~3us earlier
        b1,b2 [16,16] - big ops, minimal per-op overhead
        b3 [16,8,4,4] - the final dependency chain past the last HBM
                       byte is one 4-col sub/square/reduce (~2us)

Compute: per batch, 2 transpose groups of 16 c-columns fill [128,16,128]
PSUM tiles (4 banks, bufs=2); subs read group slices at chunk
granularity (region-level deps let an 8-col sub start after 8 of the
group's 16 transposes).  DVE sub -> ACT square -> DVE reduce over d;
each reduce is emitted after the NEXT chunk's sub so the in-order DVE
queue never head-of-line blocks on ACT.

Tail: sqrt for batches 0-2 runs while batch 3 still streams; only b3's
quarter-sqrt sits past the final reduce, and the bf16 output store is
issued from the SAME (scalar) queue as that sqrt, avoiding a
cross-engine semaphore hop before the store.
"""

import numpy as np

import concourse.bacc as bacc
import concourse.bass as bass
import concourse.mybir as mybir
import concourse.tile as tile
from concourse import masks
from concourse.bass_utils import run_bass_kernel_spmd

B, N, D = 32, 8192, 64
NCORES = 8
BL = B // NCORES        # 4 local batches per core
P = 128
NH = 2                  # n-halves per batch (partition interleave of y)
CPB = N // NH // P      # 32 consecutive x rows per partition per half
NG = 2                  # transpose groups per batch
GC = CPB // NG          # 16 columns per group
CHUNKS = {0: [8, 8, 16], 1: [16, 16], 2: [16, 16], 3: [16, 8, 4, 4]}

F32 = mybir.dt.float32
BF16 = mybir.dt.bfloat16


def _build() -> bass.Bass:
    nc = bacc.Bacc("TRN2", target_bir_lowering=False, debug=False, num_devices=NCORES)
    x_d = nc.dram_tensor("x", [BL, N, D], F32, kind="ExternalInput")
    y_d = nc.dram_tensor("y", [BL, D, N], F32, kind="ExternalInput")
    o_d = nc.dram_tensor("o", [P, BL, NG, NH, GC], BF16, kind="ExternalOutput")

    with tile.TileContext(nc) as tc:
        with (
            tc.tile_pool(name="const", bufs=1) as cpool,
            tc.tile_pool(name="io", bufs=4) as iopool,
            tc.tile_pool(name="work", bufs=4) as wpool,
            tc.tile_pool(name="res", bufs=2) as rpool,
            tc.tile_pool(name="psum", bufs=2, space="PSUM") as ppool,
        ):
            # ---- Phase 1: issue every input DMA before anything else.
            x_ts, y_ts = [], []
            for b in range(BL):
                x_t = iopool.tile([P, NH, CPB, D], F32, tag="x")
                y_t = iopool.tile([P, NH * CPB * D], F32, tag="y")
                x_ts.append(x_t)
                y_ts.append(y_t)
                nc.sync.dma_start(
                    y_t[:], y_d[b].rearrange("d (nh n) -> (d nh) n", nh=NH)
                )
                xsrc = x_d[b].rearrange("(nh q c) d -> q nh c d", nh=NH, c=CPB)
                c0 = 0
                for w in CHUNKS[b]:
                    nc.sync.dma_start(
                        x_t[:, :, c0 : c0 + w, :], xsrc[:, :, c0 : c0 + w, :]
                    )
                    c0 += w

            # ---- Phase 2: prologue constants (overlap with the stream).
            ident = cpool.tile([P, P], F32)
            masks.make_identity(nc, ident[:])
            d2a = cpool.tile([P, BL, NG, NH, GC], F32)
            warm = cpool.tile([P, 1], F32)
            nc.scalar.activation(
                warm[:], ident[:, 0:1], mybir.ActivationFunctionType.Sqrt
            )

            # ---- Phase 3: per-batch compute, reduce emitted one chunk late.
            pend = []

            def flush_reduce():
                for out_ap, sq_ap in pend:
                    nc.vector.tensor_reduce(
                        out_ap, sq_ap,
                        axis=mybir.AxisListType.X,
                        op=mybir.AluOpType.add,
                    )
                pend.clear()

            for b in range(BL):
                x_t, y_t = x_ts[b], y_ts[b]
                # column q of slice c holds n-offset q*32+c within each half
                y_v = y_t[:].rearrange("p (q c) -> p c q", c=CPB)
                chunks = CHUNKS[b]
                bounds = np.cumsum([0] + chunks).tolist()
                for g in range(NG):
                    yT = ppool.tile([P, GC, P], F32, tag="yT")
                    for c in range(GC):
                        nc.tensor.transpose(
                            yT[:, c, :], y_v[:, g * GC + c, :], ident[:]
                        )
                    yT_v = yT[:].rearrange("p c (d nh) -> p nh c d", nh=NH)

                    diff = wpool.tile([P, NH, GC, D], F32, tag="diff")
                    sq = wpool.tile([P, NH, GC, D], F32, tag="sq")
                    for c0, w in zip(bounds[:-1], chunks):
                        if not (g * GC <= c0 < (g + 1) * GC):
                            continue
                        cs = slice(c0 - g * GC, c0 - g * GC + w)
                        nc.vector.tensor_sub(
                            diff[:, :, cs, :],
                            x_t[:, :, c0 : c0 + w, :],
                            yT_v[:, :, cs, :],
                        )
                        nc.scalar.activation(
                            sq[:, :, cs, :],
                            diff[:, :, cs, :],
                            mybir.ActivationFunctionType.Square,
                        )
                        flush_reduce()
                        pend.append((d2a[:, b, g, :, cs], sq[:, :, cs, :]))

            pda = rpool.tile([P, BL, NG, NH, GC], BF16, tag="pd")
            # sqrt for batches 0..2 runs while batch 3 still streams; only
            # the last batch's sqrt sits past the final reduce
            nc.scalar.activation(
                pda[:, : BL - 1], d2a[:, : BL - 1],
                mybir.ActivationFunctionType.Sqrt,
            )
            flush_reduce()
            nc.scalar.activation(
                pda[:, BL - 1], d2a[:, BL - 1],
                mybir.ActivationFunctionType.Sqrt,
            )
            nc.scalar.dma_start(o_d[:], pda[:])
    nc.finalize()
    return nc


_NC_CACHE: list = []


def _get_program() -> bass.Bass:
    if not _NC_CACHE:
        _NC_CACHE.append(_build())
    return _NC_CACHE[0]


def kernel(x: np.ndarray, y: np.ndarray) -> np.ndarray:
    x = np.ascontiguousarray(np.asarray(x, dtype=np.float32))
    y = np.ascontiguousarray(np.asarray(y, dtype=np.float32))
    assert x.shape == (B, N, D) and y.shape == (B, D, N)

    nc = _get_program()
    in_maps = [
        {"x": x[i * BL : (i + 1) * BL], "y": y[i * BL : (i + 1) * BL]}
        for i in range(NCORES)
    ]
    res = run_bass_kernel_spmd(nc, in_maps, list(range(NCORES)))
    o = np.stack(
        [np.asarray(res.results[i]["o"], dtype=np.float32) for i in range(NCORES)]
    )  # [8, P, BL, NG, NH, GC]
    # o[core, p, b, g, nh, c] = pd[core*BL + b, nh*4096 + p*32 + g*GC + c]
    pd = o.transpose(0, 2, 4, 1, 3, 5).reshape(B, N)

    dist = pd.mean(axis=0, dtype=np.float64)
    dist[1:3] *= 1.5
    return np.asarray(dist.mean(), dtype=np.float32)


# revision 5
# speedup vs baseline: 1.0964x; 1.0964x over previous
"""Trainium2 Bass kernel for nn_EuclideanLoss (v12).

Math: pd = sqrt(sum((x - transpose(y))^2, -1)); loss = mean with [1:3]*1.5.
Data-parallel over batch: 4 batches per core; host finishes the mean in f64.
DMA-bound: 16MB input per core, ~42us at the per-core HBM limit.

    n = nh*4096 + q*32 + c      q = partition, nh = n-half, c in [0,32)

DMA plan (all input DMAs issued before the prologue ops):
  y[b]: ONE flat DMA into [128, 4096], partition 2d+nh (16KB descriptors)
  x[b]: c-chunk DMAs matching the compute chunks:
        b0 [8,8,16]  - the first sub gates on only 8 transposes + 0.5MB
                       of x, starting DVE ~3us earlier
        b1,b2 [16,16] - big ops, minimal per-op overhead
        b3 [16,8,8]  - the final dependency chain past the last HBM byte
                       is one 8-col sub/square/reduce (~2us)

Compute: per batch, 2 transpose groups of 16 c-columns fill [128,16,128]
PSUM tiles (4 banks, bufs=2); subs read group slices at chunk
granularity (region-level deps let an 8-col sub start after 8 of the
group's 16 transposes).  DVE sub -> ACT square -> DVE reduce over d;
each reduce is emitted after the NEXT chunk's sub so the in-order DVE
queue never head-of-line blocks on ACT.  Last batch: ALL subs emitted
before its reduces (subs are x-arrival-gated; queueing a 2.3us reduce
ahead of one would delay the tail).  sqrt for batches 0-2 runs while
batch 3 still streams; the bf16 output store issues from the SAME
(scalar) queue as the final quarter-sqrt.
"""

import numpy as np

import concourse.bacc as bacc
import concourse.bass as bass
import concourse.mybir as mybir
import concourse.tile as tile
from concourse import masks
from concourse.bass_utils import run_bass_kernel_spmd

B, N, D = 32, 8192, 64
NCORES = 8
BL = B // NCORES        # 4 local batches per core
P = 128
NH = 2                  # n-halves per batch (partition interleave of y)
CPB = N // NH // P      # 32 consecutive x rows per partition per half
NG = 2                  # transpose groups per batch
GC = CPB // NG          # 16 columns per group
CHUNKS = {0: [8, 8, 16], 1: [16, 16], 2: [16, 16], 3: [16, 8, 4, 4]}

F32 = mybir.dt.float32
BF16 = mybir.dt.bfloat16


def _build() -> bass.Bass:
    nc = bacc.Bacc("TRN2", target_bir_lowering=False, debug=False, num_devices=NCORES)
    x_d = nc.dram_tensor("x", [BL, N, D], F32, kind="ExternalInput")
    y_d = nc.dram_tensor("y", [BL, D, N], F32, kind="ExternalInput")
    o_d = nc.dram_tensor("o", [P, BL, NG, NH, GC], BF16, kind="ExternalOutput")

    with tile.TileContext(nc) as tc:
        with (
            tc.tile_pool(name="const", bufs=1) as cpool,
            tc.tile_pool(name="io", bufs=4) as iopool,
            tc.tile_pool(name="work", bufs=4) as wpool,
            tc.tile_pool(name="res", bufs=2) as rpool,
            tc.tile_pool(name="psum", bufs=2, space="PSUM") as ppool,
        ):
            # ---- Phase 1: issue every input DMA before anything else.
            x_ts, y_ts = [], []
            for b in range(BL):
                x_t = iopool.tile([P, NH, CPB, D], F32, tag="x")
                y_t = iopool.tile([P, NH * CPB * D], F32, tag="y")
                x_ts.append(x_t)
                y_ts.append(y_t)
                nc.sync.dma_start(
                    y_t[:], y_d[b].rearrange("d (nh n) -> (d nh) n", nh=NH)
                )
                xsrc = x_d[b].rearrange("(nh q c) d -> q nh c d", nh=NH, c=CPB)
                c0 = 0
                for w in CHUNKS[b]:
                    nc.sync.dma_start(
                        x_t[:, :, c0 : c0 + w, :], xsrc[:, :, c0 : c0 + w, :]
                    )
                    c0 += w

            # ---- Phase 2: prologue constants (overlap with the stream).
            ident = cpool.tile([P, P], F32)
            masks.make_identity(nc, ident[:])
            d2a = cpool.tile([P, BL, NG, NH, GC], F32)
            warm = cpool.tile([P, 1], F32)
            nc.scalar.activation(
                warm[:], ident[:, 0:1], mybir.ActivationFunctionType.Sqrt
            )

            # ---- Phase 3: per-batch compute, reduce emitted one chunk late.
            pend = []

            def flush_reduce():
                for out_ap, sq_ap in pend:
                    nc.vector.tensor_reduce(
                        out_ap, sq_ap,
                        axis=mybir.AxisListType.X,
                        op=mybir.AluOpType.add,
                    )
                pend.clear()

            for b in range(BL):
                x_t, y_t = x_ts[b], y_ts[b]
                if b == BL - 1:
                    # drain b2's pending reduce now; b3's reduces are all
                    # emitted AFTER its subs so each x-gated sub is never
                    # queued behind a 2.3us reduce
                    flush_reduce()
                # column q of slice c holds n-offset q*32+c within each half
                y_v = y_t[:].rearrange("p (q c) -> p c q", c=CPB)
                chunks = CHUNKS[b]
                bounds = np.cumsum([0] + chunks).tolist()
                for g in range(NG):
                    yT = ppool.tile([P, GC, P], F32, tag="yT")
                    for c in range(GC):
                        nc.tensor.transpose(
                            yT[:, c, :], y_v[:, g * GC + c, :], ident[:]
                        )
                    yT_v = yT[:].rearrange("p c (d nh) -> p nh c d", nh=NH)

                    diff = wpool.tile([P, NH, GC, D], F32, tag="diff")
                    sq = wpool.tile([P, NH, GC, D], F32, tag="sq")
                    for c0, w in zip(bounds[:-1], chunks):
                        if not (g * GC <= c0 < (g + 1) * GC):
                            continue
                        cs = slice(c0 - g * GC, c0 - g * GC + w)
                        nc.vector.tensor_sub(
                            diff[:, :, cs, :],
                            x_t[:, :, c0 : c0 + w, :],
                            yT_v[:, :, cs, :],
                        )
                        nc.scalar.activation(
                            sq[:, :, cs, :],
                            diff[:, :, cs, :],
                            mybir.ActivationFunctionType.Square,
                        )
                        if b < BL - 1:
                            flush_reduce()
                        pend.append((d2a[:, b, g, :, cs], sq[:, :, cs, :]))

            pda = rpool.tile([P, BL, NG, NH, GC], BF16, tag="pd")
            # sqrt for batches 0..2 runs while batch 3 still streams; only
            # the last batch's sqrt sits past the final reduce
            nc.scalar.activation(
                pda[:, : BL - 1], d2a[:, : BL - 1],
                mybir.ActivationFunctionType.Sqrt,
            )
            flush_reduce()
            nc.scalar.activation(
                pda[:, BL - 1], d2a[:, BL - 1],
                mybir.ActivationFunctionType.Sqrt,
            )
            nc.scalar.dma_start(o_d[:], pda[:])
    nc.finalize()
    return nc


_NC_CACHE: list = []


def _get_program() -> bass.Bass:
    if not _NC_CACHE:
        _NC_CACHE.append(_build())
    return _NC_CACHE[0]


def kernel(x: np.ndarray, y: np.ndarray) -> np.ndarray:
    x = np.ascontiguousarray(np.asarray(x, dtype=np.float32))
    y = np.ascontiguousarray(np.asarray(y, dtype=np.float32))
    assert x.shape == (B, N, D) and y.shape == (B, D, N)

    nc = _get_program()
    in_maps = [
        {"x": x[i * BL : (i + 1) * BL], "y": y[i * BL : (i + 1) * BL]}
        for i in range(NCORES)
    ]
    res = run_bass_kernel_spmd(nc, in_maps, list(range(NCORES)))
    o = np.stack(
        [np.asarray(res.results[i]["o"], dtype=np.float32) for i in range(NCORES)]
    )  # [8, P, BL, NG, NH, GC]
    # o[core, p, b, g, nh, c] = pd[core*BL + b, nh*4096 + p*32 + g*GC + c]
    pd = o.transpose(0, 2, 4, 1, 3, 5).reshape(B, N)

    dist = pd.mean(axis=0, dtype=np.float64)
    dist[1:3] *= 1.5
    return np.asarray(dist.mean(), dtype=np.float32)
